# revision 22
# baseline (speedup 1.0000x reference)
"""Trainium2 Bass kernel for nn_GAT_DSSE_BiLevel_Stable (2-layer GAT + MLP head).

Strategy: shard by DESTINATION node across 8 cores (each core owns N/8 dsts and
all their in-edges). Host reorders edges into a per-core, per-(lo,hi)-degree
bucketed ELL structure in k-major layout: slot position = k*128 + dst_lane, so
every 128-slot chunk is partition-aligned with a window of 128 destinations.
Segment max/sum become elementwise [128,4] ops; the alpha-weighted aggregate
becomes PSUM-accumulating identity matmuls; xi (dst-side features) needs no
gather. xl[src] rows are fetched with the custom GPSIMD dma_gather (int16
indices -> the 50000-node table is split into two halves < 32768 rows).
Softmax max-subtraction is skipped (logits are clipped to [-8,8] so exp is
safe); division by sum_ex is deferred to the per-window epilogue.
One AllGather of h (row-major + feature-major copies) between layers.
"""

import os
import numpy as np

NCORES = 8
P = 128
BUCKETS = [4, 8, 16, 32, 64, 128]


def _bucket(d):
    for b in BUCKETS:
        if d <= b:
            return b
    raise AssertionError(f"degree {d} exceeds max bucket")


def _build_structure(src, dst, edge_attr, N, E_DIM):
    """Build the per-core ELL structure. Returns dict of host arrays."""
    ND = N // NCORES
    NHALF = N // 2
    owner = dst // ND
    half = (src >= NHALF).astype(np.int64)

    # per (core, local dst): lo/hi degrees
    ld = dst % ND
    deg = np.zeros((NCORES, ND, 2), np.int64)
    np.add.at(deg, (owner, ld, half), 1)

    ba = np.zeros((NCORES, ND), np.int64)
    bb = np.zeros((NCORES, ND), np.int64)
    for b in reversed(BUCKETS):
        ba[deg[:, :, 0] <= b] = b
        bb[deg[:, :, 1] <= b] = b

    # joint bucket keys present anywhere
    keys = sorted({(int(a), int(b)) for a, b in
                   zip(ba.ravel(), bb.ravel())})
    # counts per core per key -> padded max (multiple of 128)
    n_max = {}
    for key in keys:
        cnt = [(int(((ba[c] == key[0]) & (bb[c] == key[1])).sum()))
               for c in range(NCORES)]
        n_max[key] = ((max(cnt) + P - 1) // P) * P
    n_pad = sum(n_max.values())
    HALF_T = 4 * n_pad
    assert HALF_T + P <= 32768, f"half table too big: {HALF_T}"

    # per-core ell order: positions for each key block
    key_off = {}
    o = 0
    for key in keys:
        key_off[key] = o
        o += n_max[key]

    node_at = np.zeros((NCORES, n_pad), np.int64)   # global node id per position
    pos_of = np.full((NCORES, ND), -1, np.int64)
    is_real = np.zeros((NCORES, n_pad), bool)
    for c in range(NCORES):
        for key in keys:
            mem = np.nonzero((ba[c] == key[0]) & (bb[c] == key[1]))[0]
            o = key_off[key]
            pos_of[c, mem] = o + np.arange(len(mem))
            node_at[c, o:o + len(mem)] = c * ND + mem
            is_real[c, o:o + len(mem)] = True
            # dummies: clone the core's first node
            node_at[c, o + len(mem):o + n_max[key]] = c * ND
    # window list (same for all cores): (key, window) in order
    windows = []
    for key in keys:
        for w in range(n_max[key] // P):
            windows.append(key)
    Wn = n_pad // P
    assert len(windows) == Wn

    # column offsets per window for idx / mask4 / ea arrays
    idx_off, m4_off, ea_off = [], [], []
    io = mo = eo = 0
    for (a, b) in windows:
        idx_off.append((io, io + a * 8))     # lo block, hi block starts
        io += (a + b) * 8
        m4_off.append(mo)
        mo += (a + b) * 4
        ea_off.append(eo)
        eo += ((a + b + 2) // 3) * P
    idx_cols, m4_cols, ea_cols = io, mo, eo

    # permuted table position of each global node (no zero-block shift)
    permpos = np.zeros(N, np.int64)
    for c in range(NCORES):
        rl = np.nonzero(pos_of[c] >= 0)[0]
        permpos[c * ND + rl] = c * n_pad + pos_of[c, rl]

    # per-edge slot math (vectorized)
    pos_e = pos_of[owner, ld]
    a_e = ba[owner, ld]
    win_e = pos_e // P
    lane_e = pos_e % P
    # occurrence rank j within (owner, ld, half)
    grp = (owner * ND + ld) * 2 + half
    order = np.argsort(grp, kind="stable")
    gs = grp[order]
    startmask = np.ones(len(gs), bool)
    startmask[1:] = gs[1:] != gs[:-1]
    gid = np.cumsum(startmask) - 1
    starts = np.nonzero(startmask)[0]
    j_sorted = np.arange(len(gs)) - starts[gid]
    j_e = np.empty(len(gs), np.int64)
    j_e[order] = j_sorted

    kall_e = np.where(half == 0, j_e, a_e + j_e)
    k_in_half = j_e

    # build per-core arrays
    idx_all = np.full((NCORES, 16, idx_cols), HALF_T, np.int16)
    m4_all = np.zeros((NCORES, P, m4_cols), np.float32)
    ea_all = np.zeros((NCORES, P, ea_cols), np.float32)

    idx_off_arr_lo = np.array([x[0] for x in idx_off], np.int64)
    idx_off_arr_hi = np.array([x[1] for x in idx_off], np.int64)
    m4_off_arr = np.array(m4_off, np.int64)
    ea_off_arr = np.array(ea_off, np.int64)

    # idx values: permuted row, hi-half minus HALF_T
    val_e = permpos[src] - half * HALF_T
    assert (val_e >= 0).all() and (val_e < HALF_T).all()
    off_e = np.where(half == 0, idx_off_arr_lo[win_e], idx_off_arr_hi[win_e])
    col_e = off_e + k_in_half * 8 + lane_e // 16
    row_e = lane_e % 16
    idx_all[owner, row_e, col_e] = val_e.astype(np.int16)

    m4c = m4_off_arr[win_e] + kall_e * 4
    for h in range(4):
        m4_all[owner, lane_e, m4c + h] = 1.0

    eac = ea_off_arr[win_e] + (kall_e // 3) * P + lane_e
    ear = 32 * (kall_e % 3)
    for d in range(E_DIM):
        ea_all[owner, ear + d, eac] = edge_attr[:, d]

    idx_all = np.tile(idx_all, (1, 8, 1))  # replicate to 128 partitions

    return dict(
        ND=ND, n_pad=n_pad, HALF_T=HALF_T, Wn=Wn, windows=windows,
        idx_off=idx_off, m4_off=m4_off, ea_off=ea_off,
        idx_cols=idx_cols, m4_cols=m4_cols, ea_cols=ea_cols,
        node_at=node_at, is_real=is_real, pos_of=pos_of,
        idx_all=idx_all, m4_all=m4_all, ea_all=ea_all,
    )


def _trace_program(S, F_IN, E_DIM, HC, DENSE, OUT, trn="TRN2"):
    import concourse.bacc as bacc
    import concourse.bass as bass
    import concourse.mybir as mybir
    import concourse.tile as tile
    from concourse.masks import make_identity
    from contextlib import ExitStack

    f32 = mybir.dt.float32
    i16 = mybir.dt.int16
    AF = mybir.ActivationFunctionType
    OP = mybir.AluOpType

    n_pad, Wn, windows = S["n_pad"], S["Wn"], S["windows"]
    MAXW = int(os.environ.get("K_MAXW", "0"))   # debug: limit edge windows
    MAXL = int(os.environ.get("K_MAXL", "2"))   # debug: limit layers
    MAXA = int(os.environ.get("K_MAXA", "0"))   # debug: limit phase-A tiles
    ONLYW = os.environ.get("K_ONLYW", "")       # debug: only these windows
    NOGATHER = int(os.environ.get("K_NOGATHER", "0"))
    NOP1K = int(os.environ.get("K_NOP1K", "0"))
    NOP23 = int(os.environ.get("K_NOP23", "0"))
    HALF_T = S["HALF_T"]
    NTAB = 8 * n_pad                      # perm-order rows (no zero blocks)
    TROWS = 2 * HALF_T + 2 * P            # gather table rows incl zero blocks
    L = 2

    nc = bacc.Bacc(trn, target_bir_lowering=False, debug=False,
                   num_devices=NCORES)

    # ---------------- dram tensors ----------------
    xpermT = nc.dram_tensor("xpermT", [F_IN, NTAB], f32, kind="ExternalInput")
    xpermT_own = nc.dram_tensor("xpermT_own", [F_IN, n_pad], f32,
                                kind="ExternalInput")
    idx_in = nc.dram_tensor("idx_all", [P, S["idx_cols"]], i16,
                            kind="ExternalInput")
    m4_in = nc.dram_tensor("m4_all", [P, S["m4_cols"]], f32,
                           kind="ExternalInput")
    ea_in = nc.dram_tensor("ea_all", [P, S["ea_cols"]], f32,
                           kind="ExternalInput")

    wnames = ["Wl0", "Wr0", "We0", "att0", "bias0", "lng0", "lnb0",
              "Wl1", "Wr1", "We1", "att1", "bias1", "lng1", "lnb1",
              "projg", "projb", "W1", "W2a", "W2b", "W3",
              "bl0", "br0", "bl1", "br1",
              "bn1s", "bn1b", "bn2s", "bn2b", "b3"]
    wshapes = {
        "Wl0": [F_IN, HC], "Wr0": [F_IN, HC], "We0": [P, HC],
        "att0": [P, HC], "bias0": [P, HC], "lng0": [P, HC], "lnb0": [P, HC],
        "Wl1": [HC, HC], "Wr1": [HC, HC], "We1": [P, HC],
        "att1": [P, HC], "bias1": [P, HC], "lng1": [P, HC], "lnb1": [P, HC],
        "projg": [P, HC], "projb": [P, HC],
        "W1": [HC, DENSE], "W2a": [HC, HC], "W2b": [HC, HC], "W3": [HC, OUT],
        "bl0": [HC, 1], "br0": [HC, 1], "bl1": [HC, 1], "br1": [HC, 1],
        "bn1s": [HC, 2], "bn1b": [HC, 2], "bn2s": [HC, 1], "bn2b": [HC, 1],
        "b3": [OUT, 1],
    }
    wdram = {n: nc.dram_tensor(n, wshapes[n], f32, kind="ExternalInput")
             for n in wnames}

    xl_tab = [nc.dram_tensor(f"xl_tab{l}", [TROWS, HC], f32) for l in range(L)]
    h1_own = nc.dram_tensor("h1_own", [n_pad, HC], f32)
    h1_full = nc.dram_tensor("h1_full", [NTAB, HC], f32, addr_space="Shared")
    h1T_own = nc.dram_tensor("h1T_own", [P, n_pad], f32)
    h1T_full = nc.dram_tensor("h1T_full", [8 * P, n_pad], f32,
                              addr_space="Shared")
    outT = nc.dram_tensor("outT", [OUT, n_pad], f32, kind="ExternalOutput")

    a_max = max(k[0] for k in windows)
    b_max = max(k[1] for k in windows)

    with ExitStack() as ctx:
        tc = ctx.enter_context(tile.TileContext(nc))
        wp = ctx.enter_context(tc.tile_pool(name="wp", bufs=1))
        xrp = ctx.enter_context(tc.tile_pool(name="xrp", bufs=1))
        ap_ = ctx.enter_context(tc.tile_pool(name="ap", bufs=3))
        gp = ctx.enter_context(tc.tile_pool(name="gp", bufs=2))
        kp = ctx.enter_context(tc.tile_pool(name="kp", bufs=3))
        sp = ctx.enter_context(tc.tile_pool(name="sp", bufs=4))
        ep = ctx.enter_context(tc.tile_pool(name="ep", bufs=2))
        psA = ctx.enter_context(tc.tile_pool(name="psA", bufs=3, space="PSUM"))
        psZ = ctx.enter_context(tc.tile_pool(name="psZ", bufs=3, space="PSUM"))
        psG = ctx.enter_context(tc.tile_pool(name="psG", bufs=2, space="PSUM"))

        # ---------- load constants ----------
        W = {}
        for n in wnames:
            t = wp.tile(wshapes[n], f32, tag=f"w_{n}")
            nc.sync.dma_start(t[:], wdram[n][:])
            W[n] = t
        ident = wp.tile([P, P], f32, tag="ident")
        make_identity(nc, ident[:])
        zrow = wp.tile([P, HC], f32, tag="zrow")
        nc.vector.memset(zrow[:], 0.0)
        cb7 = wp.tile([P, 1], f32, tag="cb7")
        nc.vector.memset(cb7[:], 7e-8)
        cb1e5 = wp.tile([P, 1], f32, tag="cb1e5")
        nc.vector.memset(cb1e5[:], 1e-5)
        # zero blocks of gather tables (both layers)
        for l in range(L):
            nc.sync.dma_start(xl_tab[l][HALF_T:HALF_T + P, :], zrow[:])
            nc.sync.dma_start(xl_tab[l][2 * HALF_T + P:, :], zrow[:])

        if MAXW or MAXL < 2 or MAXA:
            zo = wp.tile([OUT, P], f32, tag="zo")
            nc.vector.memset(zo[:], 0.0)
            for w_ in range(Wn):
                nc.sync.dma_start(outT[:, w_ * P:(w_ + 1) * P], zo[:])
        xr_sb = xrp.tile([P, Wn * HC], f32, tag="xr")
        nr_sb = xrp.tile([P, Wn * 4], f32, tag="nr")

        def ln_ops(x_ap, g_t, b_t, out_t):
            """LayerNorm over free axis (HC) of [P, HC] tile -> out_t."""
            msum = sp.tile([P, 1], f32, tag="ln_msum")
            nc.vector.tensor_reduce(msum[:], x_ap, mybir.AxisListType.X, OP.add)
            mmean = sp.tile([P, 1], f32, tag="ln_mmean")
            nc.scalar.mul(mmean[:], msum[:], 1.0 / HC)
            zm = kp.tile([P, HC], f32, tag="ln_zm")
            nc.vector.tensor_mul(zm[:], x_ap, x_ap)
            s2 = sp.tile([P, 1], f32, tag="ln_s2")
            nc.vector.tensor_reduce(s2[:], zm[:], mybir.AxisListType.X, OP.add)
            msq = sp.tile([P, 1], f32, tag="ln_msq")
            nc.scalar.square(msq[:], mmean[:])
            var = sp.tile([P, 1], f32, tag="ln_var")
            nc.vector.scalar_tensor_tensor(var[:], s2[:], 1.0 / HC, msq[:],
                                           OP.mult, OP.subtract)
            sd = sp.tile([P, 1], f32, tag="ln_sd")
            nc.scalar.activation(sd[:], var[:], AF.Sqrt, bias=cb1e5[:])
            rstd = sp.tile([P, 1], f32, tag="ln_rstd")
            nc.vector.reciprocal(rstd[:], sd[:])
            xn = kp.tile([P, HC], f32, tag="ln_xn")
            nc.vector.tensor_scalar(xn[:], x_ap, mmean[:], rstd[:],
                                    OP.subtract, OP.mult)
            xg = kp.tile([P, HC], f32, tag="ln_xg")
            nc.vector.tensor_mul(xg[:], xn[:], g_t[:])
            nc.vector.tensor_add(out_t[:], xg[:], b_t[:])

        for l in range(min(L, MAXL)):
            in_ch = F_IN if l == 0 else HC
            Wl, Wr, We = W[f"Wl{l}"], W[f"Wr{l}"], W[f"We{l}"]
            bl, br = W[f"bl{l}"], W[f"br{l}"]
            attrep, biasrep = W[f"att{l}"], W[f"bias{l}"]
            lng, lnb = W[f"lng{l}"], W[f"lnb{l}"]

            # ---------- phase A: gather table xl_tab[l] = h @ Wl + bl ----------
            for t in range(MAXA if MAXA else NTAB // P):
                rhs = ap_.tile([in_ch, P], f32, tag="pa_rhs")
                if l == 0:
                    nc.sync.dma_start(rhs[:], xpermT[:, t * P:(t + 1) * P])
                else:
                    r = (t * P) // n_pad
                    c0 = t * P - r * n_pad
                    nc.sync.dma_start(
                        rhs[:], h1T_full[r * P:(r + 1) * P, c0:c0 + P])
                pz = psA.tile([P, P], f32, tag="pa", space="PSUM")
                nc.tensor.matmul(pz[:], lhsT=Wl[:], rhs=rhs[:],
                                 start=True, stop=True)
                xlT = ap_.tile([P, P], f32, tag="pa_xlT")
                nc.scalar.activation(xlT[:], pz[:], AF.Identity, bias=bl[:])
                pz2 = psA.tile([P, P], f32, tag="pa", space="PSUM")
                nc.tensor.transpose(pz2[:], xlT[:], ident[:])
                xlrow = ap_.tile([P, P], f32, tag="pa_xlrow")
                nc.vector.tensor_copy(xlrow[:], pz2[:])
                trow = t * P + (0 if t * P < HALF_T else P)
                nc.sync.dma_start(xl_tab[l][trow:trow + P, :], xlrow[:])

            # ---------- xr windows: xr = h_own @ Wr + br; ni ----------
            for w in range(Wn):
                rhs = ap_.tile([in_ch, P], f32, tag="pa_rhs")
                if l == 0:
                    nc.sync.dma_start(rhs[:], xpermT_own[:, w * P:(w + 1) * P])
                else:
                    nc.sync.dma_start(rhs[:], h1T_own[:, w * P:(w + 1) * P])
                pz = psA.tile([P, P], f32, tag="pa", space="PSUM")
                nc.tensor.matmul(pz[:], lhsT=Wr[:], rhs=rhs[:],
                                 start=True, stop=True)
                xrT = ap_.tile([P, P], f32, tag="pa_xlT")
                nc.scalar.activation(xrT[:], pz[:], AF.Identity, bias=br[:])
                pz2 = psA.tile([P, P], f32, tag="pa", space="PSUM")
                nc.tensor.transpose(pz2[:], xrT[:], ident[:])
                nc.vector.tensor_copy(xr_sb[:, w * HC:(w + 1) * HC], pz2[:])
                # ni = sqrt(sum_c xr^2) per head
                sq = kp.tile([P, HC], f32, tag="k_sq")
                nc.scalar.square(sq[:], xr_sb[:, w * HC:(w + 1) * HC])
                nc.vector.tensor_reduce(
                    nr_sb[:, w * 4:(w + 1) * 4],
                    sq[:].rearrange("p (h c) -> p h c", h=4),
                    mybir.AxisListType.X, OP.add)
                nc.scalar.sqrt(nr_sb[:, w * 4:(w + 1) * 4],
                               nr_sb[:, w * 4:(w + 1) * 4])

            # ---------- edge windows ----------
            _wl = list(enumerate(windows))
            if MAXW:
                _wl = _wl[:MAXW]
            if ONLYW:
                _sel = {int(x) for x in ONLYW.split(",")}
                _wl = [wa for wa in _wl if wa[0] in _sel]
            for w, (a, b) in _wl:
                D = a + b
                iolo, iohi = S["idx_off"][w]
                moff, eoff = S["m4_off"][w], S["ea_off"][w]
                xjlo = gp.tile([P, a_max, HC], f32, tag="xjlo")
                xjhi = gp.tile([P, b_max, HC], f32, tag="xjhi")
                for (hf, Dh, xjb, io) in ((0, a, xjlo, iolo), (1, b, xjhi, iohi)):
                    isb = gp.tile([P, a_max * 8 if hf == 0 else b_max * 8],
                                  i16, tag=f"idx{hf}")
                    nc.sync.dma_start(isb[:, :Dh * 8],
                                      idx_in[:, io:io + Dh * 8])
                    base = 0 if hf == 0 else HALF_T + P
                    if NOGATHER:
                        nc.vector.memset(xjb[:, :Dh, :], 0.01)
                    else:
                        nc.gpsimd.dma_gather(
                            xjb[:, :Dh, :], xl_tab[l][base:base + HALF_T + P, :],
                            isb[:, :Dh * 8], Dh * P, Dh * P, HC,
                            single_packet=(Dh * P <= 1024))
                easb = gp.tile([P, ((a_max + b_max + 2) // 3) * P], f32,
                               tag="easb")
                necols = ((D + 2) // 3) * P
                nc.sync.dma_start(easb[:, :necols],
                                  ea_in[:, eoff:eoff + necols])
                m4 = gp.tile([P, (a_max + b_max) * 4], f32, tag="m4")
                nc.sync.dma_start(m4[:, :D * 4], m4_in[:, moff:moff + D * 4])

                ebuf = ep.tile([P, (a_max + b_max) * 4], f32, tag="ebuf")
                njmax = sp.tile([P, 4], f32, tag="njmax")
                xrw = xr_sb[:, w * HC:(w + 1) * HC]

                for kall in range(D if not NOP1K else 0):
                    hf = 1 if kall >= a else 0
                    k = kall - a if hf else kall
                    xjt = (xjhi if hf else xjlo)[:, k, :]
                    pz = psZ.tile([P, P], f32, tag="pz", space="PSUM")
                    bp = 32 * (kall % 3)
                    nc.tensor.matmul(
                        pz[:],
                        lhsT=easb[bp:bp + E_DIM,
                                  (kall // 3) * P:(kall // 3) * P + P],
                        rhs=We[bp:bp + E_DIM, :], start=True, stop=False)
                    nc.tensor.matmul(pz[:], lhsT=ident[:], rhs=xrw,
                                     start=False, stop=True)
                    s = kp.tile([P, P], f32, tag="k_s")
                    nc.vector.tensor_add(s[:], pz[:], xjt)
                    z = kp.tile([P, P], f32, tag="k_z")
                    nc.scalar.activation(z[:], s[:], AF.Lrelu, alpha=0.01)
                    zm = kp.tile([P, P], f32, tag="k_zm")
                    nc.vector.tensor_mul(zm[:], z[:], attrep[:])
                    nc.vector.tensor_reduce(
                        ebuf[:, kall * 4:kall * 4 + 4],
                        zm[:].rearrange("p (h c) -> p h c", h=4),
                        mybir.AxisListType.X, OP.add)
                    sq = kp.tile([P, P], f32, tag="k_sq2")
                    nc.scalar.square(sq[:], xjt)
                    if kall == 0:
                        nc.vector.tensor_reduce(
                            njmax[:], sq[:].rearrange("p (h c) -> p h c", h=4),
                            mybir.AxisListType.X, OP.add)
                    else:
                        t4 = sp.tile([P, 4], f32, tag="t4")
                        nc.vector.tensor_reduce(
                            t4[:], sq[:].rearrange("p (h c) -> p h c", h=4),
                            mybir.AxisListType.X, OP.add)
                        nc.vector.tensor_max(njmax[:], njmax[:], t4[:])

                if NOP23 or NOP1K:
                    continue
                # ---------- window stats ----------
                njs = sp.tile([P, 4], f32, tag="njs")
                nc.scalar.sqrt(njs[:], njmax[:])
                dsum = sp.tile([P, 4], f32, tag="dsum")
                nc.vector.tensor_add(dsum[:], njs[:], nr_sb[:, w * 4:w * 4 + 4])
                den = sp.tile([P, 4], f32, tag="den")
                nc.scalar.activation(den[:], dsum[:], AF.Identity,
                                     bias=cb7[:], scale=2.0)
                rden = sp.tile([P, 4], f32, tag="rden")
                nc.vector.reciprocal(rden[:], den[:])

                # ---------- pass 2: e -> exp ----------
                ev = ebuf[:, :D * 4].rearrange("p (k h) -> p h k", h=4)
                for h in range(4):
                    nc.vector.tensor_scalar(ev[:, h, :], ev[:, h, :],
                                            rden[:, h:h + 1], 8.0,
                                            OP.mult, OP.min)
                    nc.vector.tensor_scalar_max(ev[:, h, :], ev[:, h, :], -8.0)
                nc.scalar.activation(ebuf[:, :D * 4], ebuf[:, :D * 4], AF.Exp)
                exm = ep.tile([P, (a_max + b_max) * 4], f32, tag="exm")
                nc.vector.tensor_mul(exm[:, :D * 4], ebuf[:, :D * 4],
                                     m4[:, :D * 4])
                sumex = sp.tile([P, 4], f32, tag="sumex")
                nc.vector.tensor_reduce(
                    sumex[:], exm[:, :D * 4].rearrange("p (k h) -> p h k", h=4),
                    mybir.AxisListType.X, OP.add)
                nc.vector.tensor_scalar_add(sumex[:], sumex[:], 1e-16)
                rsum = sp.tile([P, 4], f32, tag="rsum")
                nc.vector.reciprocal(rsum[:], sumex[:])

                # ---------- pass 3: aggregate ----------
                pagg = psG.tile([P, P], f32, tag="pagg", space="PSUM")
                for kall in range(D):
                    hf = 1 if kall >= a else 0
                    k = kall - a if hf else kall
                    xjt = (xjhi if hf else xjlo)[:, k, :]
                    axj = kp.tile([P, P], f32, tag="k_axj")
                    for h in range(4):
                        nc.scalar.activation(
                            axj[:, h * 32:h * 32 + 32],
                            xjt[:, h * 32:h * 32 + 32], AF.Copy,
                            scale=exm[:, kall * 4 + h:kall * 4 + h + 1])
                    nc.tensor.matmul(pagg[:], lhsT=ident[:], rhs=axj[:],
                                     start=(kall == 0), stop=(kall == D - 1),
                                     skip_group_check=True)

                # ---------- epilogue ----------
                h1w = kp.tile([P, HC], f32, tag="k_h1w")
                for h in range(4):
                    nc.scalar.activation(h1w[:, h * 32:h * 32 + 32],
                                         pagg[:, h * 32:h * 32 + 32], AF.Copy,
                                         scale=rsum[:, h:h + 1])
                nc.vector.tensor_add(h1w[:], h1w[:], biasrep[:])
                hln = kp.tile([P, HC], f32, tag="k_hln")
                ln_ops(h1w[:], lng, lnb, hln)
                hout = kp.tile([P, HC], f32, tag="k_hout")
                if l == 0:
                    nc.scalar.activation(hout[:], hln[:], AF.Lrelu, alpha=0.01)
                    nc.sync.dma_start(h1_own[w * P:(w + 1) * P, :], hout[:])
                    pzt = psA.tile([P, P], f32, tag="pa", space="PSUM")
                    nc.tensor.transpose(pzt[:], hout[:], ident[:])
                    houtT = ap_.tile([P, P], f32, tag="pa_xlrow")
                    nc.vector.tensor_copy(houtT[:], pzt[:])
                    nc.sync.dma_start(h1T_own[:, w * P:(w + 1) * P], houtT[:])
                else:
                    res = kp.tile([P, HC], f32, tag="k_res")
                    nc.sync.dma_start(res[:], h1_own[w * P:(w + 1) * P, :])
                    hr = kp.tile([P, HC], f32, tag="k_hr")
                    nc.vector.scalar_tensor_tensor(hr[:], res[:], 0.1, hln[:],
                                                   OP.mult, OP.add)
                    nc.scalar.activation(hout[:], hr[:], AF.Lrelu, alpha=0.01)
                    # ---- MLP head on this window ----
                    hn = kp.tile([P, HC], f32, tag="k_hn")
                    ln_ops(hout[:], W["projg"], W["projb"], hn)
                    pzt = psA.tile([P, P], f32, tag="pa", space="PSUM")
                    nc.tensor.transpose(pzt[:], hn[:], ident[:])
                    hnT = kp.tile([P, P], f32, tag="k_hnT")
                    nc.vector.tensor_copy(hnT[:], pzt[:])
                    acts = []
                    for q in range(2):
                        p1 = psZ.tile([P, P], f32, tag="pz", space="PSUM")
                        nc.tensor.matmul(p1[:], lhsT=W["W1"][:, q * P:(q + 1) * P],
                                         rhs=hnT[:], start=True, stop=True)
                        a1 = kp.tile([P, P], f32, tag=f"k_a1{q}")
                        nc.scalar.activation(a1[:], p1[:], AF.Lrelu,
                                             bias=W["bn1b"][:, q:q + 1],
                                             scale=W["bn1s"][:, q:q + 1],
                                             alpha=0.01)
                        acts.append(a1)
                    p2 = psG.tile([P, P], f32, tag="pagg", space="PSUM")
                    nc.tensor.matmul(p2[:], lhsT=W["W2a"][:], rhs=acts[0][:],
                                     start=True, stop=False)
                    nc.tensor.matmul(p2[:], lhsT=W["W2b"][:],
                                     rhs=acts[1][:], start=False, stop=True)
                    a2 = kp.tile([P, P], f32, tag="k_a2")
                    nc.scalar.activation(a2[:], p2[:], AF.Lrelu,
                                         bias=W["bn2b"][:, 0:1],
                                         scale=W["bn2s"][:, 0:1], alpha=0.01)
                    p3 = psG.tile([OUT, P], f32, tag="pagg", space="PSUM")
                    nc.tensor.matmul(p3[:], lhsT=W["W3"][:], rhs=a2[:],
                                     start=True, stop=True)
                    ow = kp.tile([OUT, P], f32, tag="k_ow")
                    nc.scalar.activation(ow[:], p3[:], AF.Identity,
                                         bias=W["b3"][:])
                    nc.sync.dma_start(outT[:, w * P:(w + 1) * P], ow[:])

            if l == 0:
                nc.gpsimd.collective_compute(
                    "AllGather", mybir.AluOpType.bypass,
                    replica_groups=[list(range(NCORES))],
                    ins=[h1_own.ap().opt()], outs=[h1_full.ap().opt()])
                nc.gpsimd.collective_compute(
                    "AllGather", mybir.AluOpType.bypass,
                    replica_groups=[list(range(NCORES))],
                    ins=[h1T_own.ap().opt()], outs=[h1T_full.ap().opt()])
                # copy h1_full (perm rows) into xl-table-less layout? not
                # needed: layer-1 phase A reads h1T_full directly.

    nc.compile()
    return nc


def kernel(x, edge_index, edge_attr, params):
    x = np.asarray(x, np.float32)
    edge_index = np.asarray(edge_index)
    edge_attr = np.asarray(edge_attr, np.float32)

    N, F_IN = x.shape
    E, E_DIM = edge_attr.shape
    HC = params["layers"][0]["Wl"].shape[1]
    H = np.asarray(params["layers"][0]["att"]).shape[1]
    C = HC // H
    DENSE = params["proj"]["W1"].shape[1]
    OUT = params["proj"]["W3"].shape[1]

    src = np.asarray(edge_index[0], np.int64)
    dst = np.asarray(edge_index[1], np.int64)

    ckey = (N, F_IN, E, E_DIM, int(src[0]), int(dst[0]), int(src[-1]))
    cache = getattr(kernel, "_cache", None)
    if cache is not None and cache[0] == ckey:
        S, nc = cache[1], cache[2]
    else:
        S = _build_structure(src, dst, edge_attr, N, E_DIM)
        nc = _trace_program(S, F_IN, E_DIM, HC, DENSE, OUT)
        kernel._cache = (ckey, S, nc)
    n_pad, ND = S["n_pad"], S["ND"]

    # ---------------- inputs ----------------
    node_at = S["node_at"]                       # [NCORES, n_pad] global ids
    xperm = x[node_at.reshape(-1)]               # [8*n_pad, F_IN]
    xpermT = np.ascontiguousarray(xperm.T)       # [F_IN, 8*n_pad]

    def rep(v):   # [HC] -> [128, HC] replicated
        return np.tile(np.asarray(v, np.float32)[None, :], (P, 1))

    pl = params["layers"]
    pp = params["proj"]
    bn_c = np.float32(1.0 / np.sqrt(1.0 + 1e-5))
    wvals = {}
    for l in range(2):
        att = np.asarray(pl[l]["att"], np.float32).reshape(H * C)
        wvals[f"Wl{l}"] = np.asarray(pl[l]["Wl"], np.float32)
        wvals[f"Wr{l}"] = np.asarray(pl[l]["Wr"], np.float32)
        We_v = np.asarray(pl[l]["We"], np.float32)
        We_rep = np.zeros((P, HC), np.float32)
        for q in range(3):
            We_rep[32 * q:32 * q + E_DIM] = We_v
        wvals[f"We{l}"] = We_rep
        wvals[f"att{l}"] = rep(att)
        wvals[f"bias{l}"] = rep(pl[l]["bias"])
        wvals[f"lng{l}"] = rep(params["ln"][l]["g"])
        wvals[f"lnb{l}"] = rep(params["ln"][l]["b"])
        wvals[f"bl{l}"] = np.asarray(pl[l]["bl"], np.float32)[:, None]
        wvals[f"br{l}"] = np.asarray(pl[l]["br"], np.float32)[:, None]
    wvals["projg"] = rep(pp["ln_g"])
    wvals["projb"] = rep(pp["ln_b"])
    wvals["W1"] = np.asarray(pp["W1"], np.float32)
    W2 = np.asarray(pp["W2"], np.float32)
    wvals["W2a"] = np.ascontiguousarray(W2[:P])
    wvals["W2b"] = np.ascontiguousarray(W2[P:])
    wvals["W3"] = np.asarray(pp["W3"], np.float32)
    s1 = np.asarray(pp["bn1_g"], np.float32) * bn_c
    wvals["bn1s"] = s1.reshape(2, P).T.copy()
    wvals["bn1b"] = (np.asarray(pp["b1"], np.float32) * s1 +
                     np.asarray(pp["bn1_b"], np.float32)).reshape(2, P).T.copy()
    s2 = np.asarray(pp["bn2_g"], np.float32) * bn_c
    wvals["bn2s"] = s2[:, None]
    wvals["bn2b"] = (np.asarray(pp["b2"], np.float32) * s2 +
                     np.asarray(pp["bn2_b"], np.float32))[:, None]
    wvals["b3"] = np.asarray(pp["b3"], np.float32)[:, None]

    in_maps = []
    for c in range(NCORES):
        m = {
            "xpermT": xpermT,
            "xpermT_own": np.ascontiguousarray(
                xpermT[:, c * n_pad:(c + 1) * n_pad]),
            "idx_all": S["idx_all"][c],
            "m4_all": S["m4_all"][c],
            "ea_all": S["ea_all"][c],
        }
        m.update(wvals)
        in_maps.append(m)

    from concourse.bass_utils import run_bass_kernel_spmd
    trace = bool(int(os.environ.get("KERNEL_TRACE", "0")))
    res = run_bass_kernel_spmd(nc, in_maps, core_ids=list(range(NCORES)),
                               trace=trace)
    if trace and res.exec_time_ns is not None:
        print(f"HW exec time: {res.exec_time_ns} ns")
    kernel.last_results = res

    # ---------------- unshard ----------------
    out = np.zeros((N, OUT), np.float32)
    for c in range(NCORES):
        o = res.results[c]["outT"]               # [OUT, n_pad]
        real = S["is_real"][c]
        out[node_at[c][real]] = o[:, real].T
    return out
